# revision 10
# baseline (speedup 1.0000x reference)
"""Trainium2 Bass kernel for CCHead (criss-cross attention head).

Self-contained: kernel(**inputs) takes the full unsharded inputs
(x[8, 2048, 64, 64] + weights), shards batch across 8 NeuronCores
(1 image per core, all params replicated), and returns the full
output [8, 104, 64, 64] float32.

v2: all matmul operands bf16 (f32r streams 4-byte operands at 2
cycles/row on HW; bf16 runs 1 cycle/row), activations ping-pong
between two SBUF-resident padded buffer sets (no DRAM round trips
between stages), CCA restructured (paired VT/transpose matmuls,
single-pass softmax ops).
"""
import contextlib
import os

import numpy as np
import ml_dtypes

import concourse.bass as bass
import concourse.tile as tile
from concourse import bacc, mybir

V_BCAST = os.environ.get('CC_BCAST', '0') == '1'   # stride-0 broadcast normalize
V_PAIR = os.environ.get('CC_PAIR', '0') == '1'     # paired transposes/VT, base-64 MMs
V_BF16T = os.environ.get('CC_BF16T', '0') == '1'   # bf16 psum transposes (EH/EW bf16)
V_XSYNC = os.environ.get('CC_XSYNC', '1') == '1'   # x strips on sync HWDGE ring

f32 = mybir.dt.float32
bf16 = mybir.dt.bfloat16
AF = mybir.ActivationFunctionType
ALU = mybir.AluOpType

X_DEV_SHAPE = (16, 128, 64, 64)

S = 65
NR = 67
FLAT = NR * S + 2          # 4357
IMG0 = 1 + S               # flat offset of image row 0, col 0 = 66
STRIPS = [(r, 7) for r in range(0, 63, 7)] + [(63, 1)]
GROUPS = [STRIPS[0:2], STRIPS[2:4], STRIPS[4:6], STRIPS[6:8], STRIPS[8:10]]
GROUP_R0 = [0, 14, 28, 42, 56]
XS_FLAT = 16 * S + 3       # 1 lead pad + 16 rows * 65 + 2 slack
QK_TILES = [(i * 512, 512) for i in range(8)] + [(4096, 64)]
CLS_STRIPS = [(r, 7) for r in range(0, 63, 7)] + [(63, 1)]


def host_prep(inputs):
    f = np.float32
    bf = ml_dtypes.bfloat16

    def fold(w, g, b, m, v):
        s = (g / np.sqrt(v + 1e-5)).astype(f)
        return (w * s[:, None, None, None]).astype(f), (b - m * s).astype(f)

    def wt_dev(w):  # [co, ci, 3, 3] -> [nci, 128, 9, co] bf16
        co, ci = w.shape[:2]
        return np.ascontiguousarray(
            w.reshape(co, ci, 9).transpose(1, 2, 0).reshape(
                ci // 128, 128, 9, co).astype(bf))

    def t1x1(w):  # [co, ci, 1, 1] -> [nci, 128, co] bf16
        co, ci = w.shape[:2]
        return np.ascontiguousarray(
            w.reshape(co, ci).T.reshape(ci // 128, 128, co).astype(bf))

    wa, ba = fold(inputs['conva_w'], inputs['conva_g'], inputs['conva_b'],
                  inputs['conva_m'], inputs['conva_v'])
    wb, bb = fold(inputs['convb_w'], inputs['convb_g'], inputs['convb_b'],
                  inputs['convb_m'], inputs['convb_v'])
    wt, bt = fold(inputs['bott_w'], inputs['bott_g'], inputs['bott_b'],
                  inputs['bott_m'], inputs['bott_v'])
    gamma = float(np.asarray(inputs['cc_gamma']).reshape(-1)[0])
    maskd = np.zeros((64, 64), f)
    np.fill_diagonal(maskd, -1e30)
    dev = {
        'wa': wt_dev(wa), 'ba': ba.reshape(4, 128, 1),
        'wb': wt_dev(wb), 'bb': bb.reshape(4, 128, 1),
        'wt': wt_dev(wt), 'bt': bt.reshape(4, 128, 1),
        'wc': t1x1(inputs['cls_w']), 'bc': inputs['cls_b'].astype(f).reshape(104, 1),
        'wq': t1x1(inputs['q_w']), 'bq': inputs['q_b'].astype(f).reshape(64, 1),
        'wk': t1x1(inputs['k_w']), 'bk': inputs['k_b'].astype(f).reshape(64, 1),
        'wv': t1x1(inputs['v_w']),
        'gvb': (gamma * inputs['v_b']).astype(f).reshape(4, 128, 1),
        'maskd': maskd.astype(bf),
        'zeros': np.zeros((128, 1056), bf),
        'ident': np.eye(64, dtype=bf),
        'identf': np.eye(64, dtype=f),
    }
    return dev, gamma


INPUT_SPECS = [
    ('wa', [16, 128, 9, 512], bf16), ('ba', [4, 128, 1], f32),
    ('wb', [4, 128, 9, 512], bf16), ('bb', [4, 128, 1], f32),
    ('wt', [20, 128, 9, 512], bf16), ('bt', [4, 128, 1], f32),
    ('wc', [4, 128, 104], bf16), ('bc', [104, 1], f32),
    ('wq', [4, 128, 64], bf16), ('bq', [64, 1], f32),
    ('wk', [4, 128, 64], bf16), ('bk', [64, 1], f32),
    ('wv', [4, 128, 512], bf16),
    ('gvb', [4, 128, 1], f32),
    ('maskd', [64, 64], bf16),
    ('zeros', [128, 1056], bf16),
    ('ident', [64, 64], bf16),
    ('identf', [64, 64], f32),
]


def build(gamma, n_reps=1, debug=False):
    nc = bacc.Bacc("TRN2", num_devices=8)
    t = {'x': nc.dram_tensor("x", list(X_DEV_SHAPE), bf16, kind="ExternalInput")}
    for nm, shape, dt in INPUT_SPECS:
        t[nm] = nc.dram_tensor(nm, shape, dt, kind="ExternalInput")
    y = nc.dram_tensor("y", [104, 64, 64], f32, kind="ExternalOutput")
    if debug:
        for nm in ['o_a', 'o_c1', 'o_c2', 'o_b', 'o_t']:
            t[nm] = nc.dram_tensor(nm, [4, 128, 64, 64], f32,
                                   kind="ExternalOutput")
    with tile.TileContext(nc) as tc:
        _build_body(tc, t, y, gamma, n_reps, debug)
    nc.compile()
    return nc


def _rows(flat_tile):
    """[128, FLAT] -> padded row view [128, 67, 65] (skips lead pad elem)."""
    return flat_tile[:, 1:1 + NR * S].rearrange("p (r c) -> p r c", c=S)


def _dump(C, dram4, blocks):
    nc = C['nc']
    for cb in range(4):
        stg = C['dbgp'].tile([128, 64, 64], f32, tag="dbg")
        nc.vector.tensor_copy(stg[:], _rows(blocks[cb])[:, 1:65, 0:64])
        nc.sync.dma_start(dram4[cb], stg[:])


def _build_body(tc, t, y, gamma, n_reps, debug):
    nc = tc.nc
    with contextlib.ExitStack() as est:
        cp = est.enter_context(tc.tile_pool(name="const", bufs=1))
        zeros = cp.tile([128, 1056], bf16)
        nc.sync.dma_start(zeros[:], t['zeros'][:])
        ident = cp.tile([64, 64], bf16)
        nc.sync.dma_start(ident[:], t['ident'][:])
        maskd = cp.tile([64, 64], bf16)
        nc.sync.dma_start(maskd[:], t['maskd'][:])
        identf = cp.tile([64, 64], f32)
        nc.sync.dma_start(identf[:], t['identf'][:])

        def load_blocks(nm, n, shape, dt=f32):
            out = []
            for i in range(n):
                tl = cp.tile(shape, dt, tag=f"{nm}{i}", name=f"{nm}{i}")
                nc.sync.dma_start(tl[:], t[nm][i])
                out.append(tl)
            return out

        C = dict(nc=nc, tc=tc, t=t, y=y, gamma=gamma, zeros=zeros, ident=ident,
                 identf=identf,
                 maskd=maskd,
                 bias_a=load_blocks('ba', 4, [128, 1]),
                 bias_b=load_blocks('bb', 4, [128, 1]),
                 bias_t=load_blocks('bt', 4, [128, 1]),
                 gvb=load_blocks('gvb', 4, [128, 1]),
                 wq=load_blocks('wq', 4, [128, 64], bf16),
                 wk=load_blocks('wk', 4, [128, 64], bf16),
                 wv=load_blocks('wv', 4, [128, 512], bf16),
                 wc=load_blocks('wc', 4, [128, 104], bf16),
                 debug=debug)
        for nm, p in [('bq', 64), ('bk', 64), ('bc', 104)]:
            C[nm] = cp.tile([p, 1], f32, tag=nm, name=nm)
            nc.sync.dma_start(C[nm][:], t[nm][:])

        ap = est.enter_context(tc.tile_pool(name="actp", bufs=1))
        A = [ap.tile([128, FLAT], bf16, tag=f"A{i}", name=f"A{i}") for i in range(4)]
        B = [ap.tile([128, FLAT], bf16, tag=f"B{i}", name=f"B{i}") for i in range(4)]
        for blk in A + B:
            _zero_act_borders(nc, blk, zeros)
        C['A'], C['B'] = A, B

        # 3 persistent x-strip staging tiles (borders pre-zeroed once)
        xsp = est.enter_context(tc.tile_pool(name="xsp", bufs=1))
        xs_tiles = []
        for i in range(3):
            xs = xsp.tile([128, XS_FLAT], bf16, tag=f"xs{i}", name=f"xs{i}")
            rv = xs[:, 1:1 + 16 * S].rearrange("p (r c) -> p r c", c=S)
            nc.sync.dma_start(xs[:, 0:1], zeros[:, 0:1])
            nc.sync.dma_start(xs[:, XS_FLAT - 2:XS_FLAT], zeros[:, 0:2])
            nc.sync.dma_start(rv[:, :, 64:65], zeros[:, 0:16].unsqueeze(2))
            xs_tiles.append(xs)
        C['xs_tiles'] = xs_tiles

        if debug:
            C['dbgp'] = est.enter_context(tc.tile_pool(name="dbgp", bufs=1))

        for _ in range(n_reps):
            _network(C)


def _zero_act_borders(nc, blk, zeros):
    rv = _rows(blk)
    nc.sync.dma_start(blk[:, 0:1], zeros[:, 0:1])               # lead pad
    nc.sync.dma_start(blk[:, FLAT - 1:FLAT], zeros[:, 0:1])     # slack
    nc.sync.dma_start(rv[:, :, 64:65], zeros[:, 0:NR].unsqueeze(2))
    nc.sync.dma_start(rv[:, 0:1, 0:64], zeros[:, 0:64].unsqueeze(1))
    nc.sync.dma_start(rv[:, 65:67, 0:64],
                      zeros[:, 0:128].rearrange("p (r c) -> p r c", c=64))


def _network(C):
    nc, tc, t = C['nc'], C['tc'], C['t']
    A, B = C['A'], C['B']
    # conva: x strips -> A
    with contextlib.ExitStack() as es:
        wp = es.enter_context(tc.tile_pool(name="wp", bufs=3))
        cps = es.enter_context(tc.tile_pool(name="cps", bufs=1, space="PSUM"))
        xg = _x_strip_getter(C)
        _conv3x3(C, wp, cps, xg, 16, t['wa'], C['bias_a'], A)
    if C['debug']:
        _dump(C, t['o_a'], A)
    # CCA 1: A -> B
    _cca(C, A, B)
    if C['debug']:
        _dump(C, t['o_c1'], B)
    # CCA 2: B -> A
    _cca(C, B, A)
    if C['debug']:
        _dump(C, t['o_c2'], A)
    # convb: A -> B
    with contextlib.ExitStack() as es:
        wp = es.enter_context(tc.tile_pool(name="wpb", bufs=3))
        cps = es.enter_context(tc.tile_pool(name="cpsb", bufs=1, space="PSUM"))
        sg = _act_src_getter(A)
        _conv3x3(C, wp, cps, sg, 4, t['wb'], C['bias_b'], B)
    if C['debug']:
        _dump(C, t['o_b'], B)
    # bott: x strips (16cb) + B (4cb) -> A
    with contextlib.ExitStack() as es:
        wp = es.enter_context(tc.tile_pool(name="wpt", bufs=3))
        cps = es.enter_context(tc.tile_pool(name="cpst", bufs=1, space="PSUM"))
        xg = _x_strip_getter(C)
        sg = _act_src_getter(B)

        def src_get(g, cb):
            return xg(g, cb) if cb < 16 else sg(g, cb - 16)

        _conv3x3(C, wp, cps, src_get, 20, t['wt'], C['bias_t'], A)
    if C['debug']:
        _dump(C, t['o_t'], A)
    # cls: A -> y
    with contextlib.ExitStack() as es:
        cop = es.enter_context(tc.tile_pool(name="cop", bufs=1))
        cpp = es.enter_context(tc.tile_pool(name="cpp", bufs=2, space="PSUM"))
        out_sb = cop.tile([104, 64, 64], f32)
        for r0, nr in CLS_STRIPS:
            n = nr * S + 1
            ps = cpp.tile([104, n], f32, tag="clsps")
            for cb in range(4):
                rhs = A[cb][:, IMG0 + r0 * S:IMG0 + r0 * S + n]
                nc.tensor.matmul(ps[:], C['wc'][cb][:], rhs,
                                 start=(cb == 0), stop=(cb == 3))
            pv = ps[:, 0:nr * S].rearrange("p (r c) -> p r c", c=S)[:, :, 0:64]
            nc.scalar.activation(out_sb[:, r0:r0 + nr, :], pv, AF.Identity,
                                 bias=C['bc'][:], scale=1.0)
        nc.sync.dma_start(C['y'][:], out_sb[:])


def _x_strip_getter(C):
    """Rotating x-strip loader: 1 big row-load per (g, cb) + edge zeroing."""
    nc, zeros, t = C['nc'], C['zeros'], C['t']
    tiles = C['xs_tiles']
    state = {'i': 0}
    cache = {}

    def get(g, cb):
        key = (g, cb)
        if key in cache:
            return cache[key]
        xs = tiles[state['i'] % 3]
        state['i'] += 1
        rv = xs[:, 1:1 + 16 * S].rearrange("p (r c) -> p r c", c=S)
        r0g = GROUP_R0[g]
        lo = max(0, r0g - 1)
        hi = min(64, r0g + 15)
        l0, l1 = lo - (r0g - 1), lo - (r0g - 1) + hi - lo
        eng = nc.sync if V_XSYNC else nc.scalar
        if l0 > 0:
            eng.dma_start(rv[:, 0:l0, 0:64],
                          zeros[:, 0:l0 * 64].rearrange("p (r c) -> p r c", c=64))
        if l1 < 16:
            eng.dma_start(rv[:, l1:16, 0:64],
                          zeros[:, 0:(16 - l1) * 64].rearrange("p (r c) -> p r c", c=64))
        eng.dma_start(rv[:, l0:l1, 0:64], t['x'][cb][:, lo:hi, :])
        res = (xs, lambda r0, _g=r0g: r0 - _g + 1)
        cache[key] = res
        # only keep entries for the current group alive in the rotation
        for k in list(cache):
            if k[0] != g:
                del cache[k]
        return res

    return get


def _act_src_getter(blocks):
    def get(g, cb):
        return (blocks[cb], lambda r0: r0 + 1)
    return get


def _conv3x3(C, wp, cps, src_getter, n_cb, w_dram, bias_sb, dst_set):
    nc = C['nc']
    for g, strips in enumerate(GROUPS):
        psums = {}
        for si, (r0, nr) in enumerate(strips):
            for co in range(4):
                psums[(si, co)] = cps.tile([128, nr * S + 1], f32,
                                           tag=f"c{si}{co}", name=f"c{si}{co}")
        for cb in range(n_cb):
            wtl = wp.tile([128, 9, 512], bf16, tag="w")
            nc.sync.dma_start(wtl[:], w_dram[cb])
            sflat, base_row = src_getter(g, cb)
            for tap in range(9):
                dy, dx = tap // 3 - 1, tap % 3 - 1
                for co in range(4):
                    for si, (r0, nr) in enumerate(strips):
                        n = nr * S + 1
                        off = 1 + (base_row(r0) + dy) * S + dx
                        nc.tensor.matmul(
                            psums[(si, co)][:],
                            wtl[:, tap, co * 128:(co + 1) * 128],
                            sflat[:, off:off + n],
                            start=(cb == 0 and tap == 0),
                            stop=(cb == n_cb - 1 and tap == 8))
        for si, (r0, nr) in enumerate(strips):
            for co in range(4):
                ps = psums[(si, co)]
                pv = ps[:, 0:nr * S].rearrange("p (r c) -> p r c", c=S)[:, :, 0:64]
                dst = _rows(dst_set[co])[:, 1 + r0:1 + r0 + nr, 0:64]
                nc.scalar.activation(dst, pv, AF.Relu, bias=bias_sb[co], scale=1.0)


def _cca(C, SRC, DST):
    """One criss-cross attention: DST = gamma*(outh+outw+v_b) + SRC."""
    nc, tc = C['nc'], C['tc']
    gamma, ident, maskd = C['gamma'], C['ident'], C['maskd']
    edt = bf16 if V_BF16T else f32
    eident = ident if V_BF16T else C['identf']
    with contextlib.ExitStack() as es:
        qkp = es.enter_context(tc.tile_pool(name="qkp", bufs=1))
        atp = es.enter_context(tc.tile_pool(name="atp", bufs=1))
        smp = es.enter_context(tc.tile_pool(name="smp", bufs=1))
        esA = es.enter_context(contextlib.ExitStack())
        psQ = esA.enter_context(tc.tile_pool(name="psQ", bufs=2, space="PSUM"))
        psE = esA.enter_context(tc.tile_pool(name="psE", bufs=2, space="PSUM"))

        q_sb = qkp.tile([64, 64, 65], bf16, tag="q")
        k_sb = qkp.tile([64, 64, 65], bf16, tag="k")
        # ---- q/k 1x1 convs
        for dst_sb, wgt, bias in [(q_sb, C['wq'], C['bq']),
                                  (k_sb, C['wk'], C['bk'])]:
            dflat = dst_sb[:].rearrange("p r c -> p (r c)")
            for off, n in QK_TILES:
                ps = psQ.tile([64, 512], f32, tag="qkps")
                for cb in range(4):
                    rhs = SRC[cb][:, IMG0 + off:IMG0 + off + n]
                    nc.tensor.matmul(ps[:, 0:n], wgt[cb][:], rhs,
                                     start=(cb == 0), stop=(cb == 3))
                nc.scalar.activation(dflat[:, off:off + n], ps[:, 0:n],
                                     AF.Identity, bias=bias[:], scale=1.0)
        # ---- energies + exp + per-slice sums
        # EH[h, w, j] (attention over height, per column w), diag-masked.
        # EW[w, h, j] (attention over width, per row h).
        EH = smp.tile([64, 64, 64], edt, tag="EH")
        EW = smp.tile([64, 64, 64], edt, tag="EW")
        ZH = smp.tile([64, 64], f32, tag="ZH")    # [h, w] sum_j exp(eh)
        ZW = smp.tile([64, 64], f32, tag="ZW")    # [w, h] sum_j exp(ew)
        for c0 in range(0, 64, 16):
            pe = psE.tile([64, 16, 64], f32, tag="pe")
            for wi in range(16):
                w = c0 + wi
                nc.tensor.matmul(pe[:, wi, :], q_sb[:, :, w], k_sb[:, :, w],
                                 start=True, stop=False)
                nc.tensor.matmul(pe[:, wi, :], ident[:], maskd[:],
                                 start=False, stop=True)
            nc.scalar.activation(EH[:, c0:c0 + 16, :], pe[:], AF.Exp)
            nc.vector.tensor_reduce(ZH[:, c0:c0 + 16], EH[:, c0:c0 + 16, :],
                                    mybir.AxisListType.X, ALU.add)
        for c0 in range(0, 64, 16):
            pe = psE.tile([64, 16, 64], f32, tag="pe")
            for hi in range(16):
                h = c0 + hi
                nc.tensor.matmul(pe[:, hi, :], q_sb[:, h, 0:64],
                                 k_sb[:, h, 0:64], start=True, stop=True)
            nc.scalar.activation(EW[:, c0:c0 + 16, :], pe[:], AF.Exp)
            nc.vector.tensor_reduce(ZW[:, c0:c0 + 16], EW[:, c0:c0 + 16, :],
                                    mybir.AxisListType.X, ALU.add)
        esA.close()
        esB = es.enter_context(contextlib.ExitStack())
        psZ = esB.enter_context(tc.tile_pool(name="psZ", bufs=1, space="PSUM"))
        psT = esB.enter_context(tc.tile_pool(name="psT", bufs=3, space="PSUM"))
        # ---- joint normalizers: R[h, w] = 1/(ZH + ZW^T), R2 = R^T
        ZWT = psZ.tile([64, 64], f32, tag="zt")
        nc.tensor.transpose(ZWT[:], ZW[:], C['identf'][:])
        R = smp.tile([64, 64], f32, tag="R")
        nc.vector.tensor_tensor(R[:], ZH[:], ZWT[:], ALU.add)
        nc.vector.reciprocal(R[:], R[:])
        RT = psZ.tile([64, 64], f32, tag="zt2")
        nc.tensor.transpose(RT[:], R[:], C['identf'][:])
        R2 = smp.tile([64, 64], f32, tag="R2")
        nc.vector.tensor_copy(R2[:], RT[:])
        # ---- normalize in place
        if V_BCAST:
            for c0 in range(0, 64, 16):
                nc.vector.tensor_tensor(
                    EH[:, c0:c0 + 16, :], EH[:, c0:c0 + 16, :],
                    R[:, c0:c0 + 16].unsqueeze(2).broadcast_to([64, 16, 64]),
                    ALU.mult)
                nc.vector.tensor_tensor(
                    EW[:, c0:c0 + 16, :], EW[:, c0:c0 + 16, :],
                    R2[:, c0:c0 + 16].unsqueeze(2).broadcast_to([64, 16, 64]),
                    ALU.mult)
        else:
            for u in range(64):
                nc.vector.tensor_scalar_mul(EH[:, u, :], EH[:, u, :],
                                            R[:, u:u + 1])
                nc.vector.tensor_scalar_mul(EW[:, u, :], EW[:, u, :],
                                            R2[:, u:u + 1])
        # ---- transposes
        if V_PAIR:
            # ATh[128=(par,j), 32, 64h], ATw[128, 32, 64w]
            ATh = atp.tile([128, 32, 64], bf16, tag="ATh")
            ATw = atp.tile([128, 32, 64], bf16, tag="ATw")
            for E, AT in [(EH, ATh), (EW, ATw)]:
                for p in range(32):
                    pst = psT.tile([128, 64], edt, tag="pt")
                    nc.tensor.transpose(
                        pst[:],
                        E[:, 2 * p:2 * p + 2, :].rearrange("p a b -> p (a b)"),
                        eident[:])
                    nc.scalar.activation(AT[:, p, :], pst[:], AF.Copy)
        else:
            ATh = atp.tile([64, 64, 64], bf16, tag="ATh")
            ATw = atp.tile([64, 64, 64], bf16, tag="ATw")
            for E, AT in [(EH, ATh), (EW, ATw)]:
                for u in range(64):
                    pst = psT.tile([64, 64], edt, tag="pt")
                    nc.tensor.transpose(pst[:], E[:, u, :], eident[:])
                    nc.scalar.activation(AT[:, u, :], pst[:], AF.Copy)
        esB.close()
        # ---- apply: w-phase (out_h) then h-phase (out_w)
        with contextlib.ExitStack() as esC:
            vtp = esC.enter_context(tc.tile_pool(name="vtp", bufs=6))
            psV = esC.enter_context(tc.tile_pool(name="psV", bufs=3, space="PSUM"))
            psO = esC.enter_context(tc.tile_pool(name="psO", bufs=4, space="PSUM"))
            for phase in range(2):  # 0: w-phase (per-column), 1: h-phase (per-row)
                AT = ATh if phase == 0 else ATw
                for c4 in range(16):  # chunks of 4 columns/rows
                    vts = []
                    if V_PAIR:
                        for pr in range(2):
                            u0 = c4 * 4 + pr * 2
                            pv = psV.tile([128, 512], f32, tag="pv")
                            for cb in range(4):
                                rv = _rows(SRC[cb])
                                # stationary operand needs one contiguous
                                # free dim: stage the (pair, 64) slab first
                                vstg = vtp.tile([128, 2, 64], bf16, tag="vstg")
                                if phase == 0:
                                    nc.vector.tensor_copy(
                                        vstg[:],
                                        rv[:, 1:65, u0:u0 + 2].rearrange(
                                            "p r w -> p w r"))
                                else:
                                    nc.vector.tensor_copy(
                                        vstg[:], rv[:, 1 + u0:3 + u0, 0:64])
                                nc.tensor.matmul(
                                    pv[:], vstg[:].rearrange("p a b -> p (a b)"),
                                    C['wv'][cb][:],
                                    start=(cb == 0), stop=(cb == 3))
                            vt = vtp.tile([128, 512], bf16, tag="vt")
                            nc.scalar.activation(vt[:], pv[:], AF.Copy)
                            vts.append(vt)
                    else:
                        for i in range(4):
                            u = c4 * 4 + i
                            pv = psV.tile([64, 512], f32, tag="pv")
                            for cb in range(4):
                                rv = _rows(SRC[cb])
                                lhsT = (rv[:, 1:65, u] if phase == 0
                                        else rv[:, 1 + u, 0:64])
                                nc.tensor.matmul(pv[:], lhsT, C['wv'][cb][:],
                                                 start=(cb == 0), stop=(cb == 3))
                            vt = vtp.tile([64, 512], bf16, tag="vt")
                            nc.scalar.activation(vt[:], pv[:], AF.Copy)
                            vts.append(vt)
                    for cbo in range(4):
                        po = psO.tile([128, 4, 64], f32, tag="po")
                        for i in range(4):
                            if V_PAIR:
                                pr, par = i // 2, (i % 2) * 64
                                lhsT = vts[pr][par:par + 64,
                                               cbo * 128:(cbo + 1) * 128]
                                rhs = AT[par:par + 64, c4 * 2 + pr, :]
                            else:
                                lhsT = vts[i][:, cbo * 128:(cbo + 1) * 128]
                                rhs = AT[:, c4 * 4 + i, :]
                            nc.tensor.matmul(po[:, i, :], lhsT, rhs,
                                             start=True, stop=True)
                        rvD = _rows(DST[cbo])
                        rvS = _rows(SRC[cbo])
                        if phase == 0:
                            o_sl = rvD[:, 1:65, c4 * 4:c4 * 4 + 4].rearrange(
                                "p h w -> p w h")
                            i_sl = rvS[:, 1:65, c4 * 4:c4 * 4 + 4].rearrange(
                                "p h w -> p w h")
                            nc.vector.scalar_tensor_tensor(
                                o_sl, po[:], gamma, i_sl, ALU.mult, ALU.add)
                        else:
                            o_sl = rvD[:, 1 + c4 * 4:5 + c4 * 4, 0:64]
                            nc.vector.scalar_tensor_tensor(
                                o_sl, po[:], gamma, o_sl, ALU.mult, ALU.add)
        # ---- + gamma * v_b (joint softmax sums to 1 across both branches)
        for cbo in range(4):
            o_in = _rows(DST[cbo])[:, 1:65, 0:64]
            nc.vector.tensor_scalar_add(o_in, o_in, C['gvb'][cbo][:])


_BUILD_CACHE = {}


def _get_nc(gamma):
    key = round(float(gamma), 12)
    if key not in _BUILD_CACHE:
        _BUILD_CACHE[key] = build(gamma, n_reps=1)
    return _BUILD_CACHE[key]


def kernel(**inputs):
    from concourse.bass_utils import run_bass_kernel_spmd
    inputs_np = {k: np.asarray(v) for k, v in inputs.items()}
    dev, gamma = host_prep(inputs_np)
    nc = _get_nc(gamma)
    xbf = inputs_np['x'].astype(ml_dtypes.bfloat16)
    in_maps = []
    for core in range(8):
        m = dict(dev)
        m['x'] = np.ascontiguousarray(xbf[core].reshape(*X_DEV_SHAPE))
        in_maps.append(m)
    res = run_bass_kernel_spmd(nc, in_maps, core_ids=list(range(8)))
    out = np.stack([r['y'].reshape(104, 64, 64) for r in res.results])
    return out.astype(np.float32)


# revision 12
# speedup vs baseline: 1.0041x; 1.0041x over previous
"""Trainium2 Bass kernel for CCHead (criss-cross attention head).

Self-contained: kernel(**inputs) takes the full unsharded inputs
(x[8, 2048, 64, 64] + weights), shards batch across 8 NeuronCores
(1 image per core, all params replicated), and returns the full
output [8, 104, 64, 64] float32.

v2: all matmul operands bf16 (f32r streams 4-byte operands at 2
cycles/row on HW; bf16 runs 1 cycle/row), activations ping-pong
between two SBUF-resident padded buffer sets (no DRAM round trips
between stages), CCA restructured (paired VT/transpose matmuls,
single-pass softmax ops).
"""
import contextlib
import os

import numpy as np
import ml_dtypes

import concourse.bass as bass
import concourse.tile as tile
from concourse import bacc, mybir

V_BCAST = os.environ.get('CC_BCAST', '1') == '1'   # stride-0 broadcast normalize
# base-64 tile_position matmuls (paired transposes/VT) crash the device
# (NRT_EXEC_UNIT_UNRECOVERABLE); keep the per-column path.
V_PAIR = os.environ.get('CC_PAIR', '0') == '1'
V_BF16T = os.environ.get('CC_BF16T', '1') == '1'   # bf16 psum transposes (EH/EW bf16)
V_XSYNC = os.environ.get('CC_XSYNC', '1') == '1'   # x strips on sync HWDGE ring

f32 = mybir.dt.float32
bf16 = mybir.dt.bfloat16
AF = mybir.ActivationFunctionType
ALU = mybir.AluOpType

X_DEV_SHAPE = (16, 128, 64, 64)

S = 65
NR = 67
FLAT = NR * S + 2          # 4357
IMG0 = 1 + S               # flat offset of image row 0, col 0 = 66
STRIPS = [(r, 7) for r in range(0, 56, 7)] + [(56, 4), (60, 4)]
GROUPS = [STRIPS[0:2], STRIPS[2:4], STRIPS[4:6], STRIPS[6:8], STRIPS[8:10]]
GROUP_R0 = [0, 14, 28, 42, 56]
XS_FLAT = 16 * S + 3       # 1 lead pad + 16 rows * 65 + 2 slack
QK_TILES = [(i * 512, 512) for i in range(8)] + [(4096, 64)]
CLS_STRIPS = [(r, 7) for r in range(0, 56, 7)] + [(56, 4), (60, 4)]


def host_prep(inputs):
    f = np.float32
    bf = ml_dtypes.bfloat16

    def fold(w, g, b, m, v):
        s = (g / np.sqrt(v + 1e-5)).astype(f)
        return (w * s[:, None, None, None]).astype(f), (b - m * s).astype(f)

    def wt_dev(w):  # [co, ci, 3, 3] -> [nci, 128, 9, co] bf16
        co, ci = w.shape[:2]
        return np.ascontiguousarray(
            w.reshape(co, ci, 9).transpose(1, 2, 0).reshape(
                ci // 128, 128, 9, co).astype(bf))

    def t1x1(w):  # [co, ci, 1, 1] -> [nci, 128, co] bf16
        co, ci = w.shape[:2]
        return np.ascontiguousarray(
            w.reshape(co, ci).T.reshape(ci // 128, 128, co).astype(bf))

    wa, ba = fold(inputs['conva_w'], inputs['conva_g'], inputs['conva_b'],
                  inputs['conva_m'], inputs['conva_v'])
    wb, bb = fold(inputs['convb_w'], inputs['convb_g'], inputs['convb_b'],
                  inputs['convb_m'], inputs['convb_v'])
    wt, bt = fold(inputs['bott_w'], inputs['bott_g'], inputs['bott_b'],
                  inputs['bott_m'], inputs['bott_v'])
    gamma = float(np.asarray(inputs['cc_gamma']).reshape(-1)[0])
    maskd = np.zeros((64, 64), f)
    np.fill_diagonal(maskd, -1e30)
    dev = {
        'wa': wt_dev(wa), 'ba': ba.reshape(4, 128, 1),
        'wb': wt_dev(wb), 'bb': bb.reshape(4, 128, 1),
        'wt': wt_dev(wt), 'bt': bt.reshape(4, 128, 1),
        'wc': t1x1(inputs['cls_w']), 'bc': inputs['cls_b'].astype(f).reshape(104, 1),
        'wq': t1x1(inputs['q_w']), 'bq': inputs['q_b'].astype(f).reshape(64, 1),
        'wk': t1x1(inputs['k_w']), 'bk': inputs['k_b'].astype(f).reshape(64, 1),
        'wv': t1x1(inputs['v_w']),
        'gvb': (gamma * inputs['v_b']).astype(f).reshape(4, 128, 1),
        'maskd': maskd.astype(bf),
        'zeros': np.zeros((128, 1056), bf),
        'ident': np.eye(64, dtype=bf),
        'identf': np.eye(64, dtype=f),
    }
    return dev, gamma


INPUT_SPECS = [
    ('wa', [16, 128, 9, 512], bf16), ('ba', [4, 128, 1], f32),
    ('wb', [4, 128, 9, 512], bf16), ('bb', [4, 128, 1], f32),
    ('wt', [20, 128, 9, 512], bf16), ('bt', [4, 128, 1], f32),
    ('wc', [4, 128, 104], bf16), ('bc', [104, 1], f32),
    ('wq', [4, 128, 64], bf16), ('bq', [64, 1], f32),
    ('wk', [4, 128, 64], bf16), ('bk', [64, 1], f32),
    ('wv', [4, 128, 512], bf16),
    ('gvb', [4, 128, 1], f32),
    ('maskd', [64, 64], bf16),
    ('zeros', [128, 1056], bf16),
    ('ident', [64, 64], bf16),
    ('identf', [64, 64], f32),
]


def build(gamma, n_reps=1, debug=False):
    nc = bacc.Bacc("TRN2", num_devices=8)
    t = {'x': nc.dram_tensor("x", list(X_DEV_SHAPE), bf16, kind="ExternalInput")}
    for nm, shape, dt in INPUT_SPECS:
        t[nm] = nc.dram_tensor(nm, shape, dt, kind="ExternalInput")
    y = nc.dram_tensor("y", [104, 64, 64], f32, kind="ExternalOutput")
    if debug:
        for nm in ['o_a', 'o_c1', 'o_c2', 'o_b', 'o_t']:
            t[nm] = nc.dram_tensor(nm, [4, 128, 64, 64], f32,
                                   kind="ExternalOutput")
    with tile.TileContext(nc) as tc:
        _build_body(tc, t, y, gamma, n_reps, debug)
    nc.compile()
    return nc


def _rows(flat_tile):
    """[128, FLAT] -> padded row view [128, 67, 65] (skips lead pad elem)."""
    return flat_tile[:, 1:1 + NR * S].rearrange("p (r c) -> p r c", c=S)


def _dump(C, dram4, blocks):
    nc = C['nc']
    for cb in range(4):
        stg = C['dbgp'].tile([128, 64, 64], f32, tag="dbg")
        nc.vector.tensor_copy(stg[:], _rows(blocks[cb])[:, 1:65, 0:64])
        nc.sync.dma_start(dram4[cb], stg[:])


def _build_body(tc, t, y, gamma, n_reps, debug):
    nc = tc.nc
    with contextlib.ExitStack() as est:
        cp = est.enter_context(tc.tile_pool(name="const", bufs=1))
        zeros = cp.tile([128, 1056], bf16)
        nc.sync.dma_start(zeros[:], t['zeros'][:])
        ident = cp.tile([64, 64], bf16)
        nc.sync.dma_start(ident[:], t['ident'][:])
        maskd = cp.tile([64, 64], bf16)
        nc.sync.dma_start(maskd[:], t['maskd'][:])
        identf = cp.tile([64, 64], f32)
        nc.sync.dma_start(identf[:], t['identf'][:])

        def load_blocks(nm, n, shape, dt=f32):
            out = []
            for i in range(n):
                tl = cp.tile(shape, dt, tag=f"{nm}{i}", name=f"{nm}{i}")
                nc.sync.dma_start(tl[:], t[nm][i])
                out.append(tl)
            return out

        C = dict(nc=nc, tc=tc, t=t, y=y, gamma=gamma, zeros=zeros, ident=ident,
                 identf=identf,
                 maskd=maskd,
                 bias_a=load_blocks('ba', 4, [128, 1]),
                 bias_b=load_blocks('bb', 4, [128, 1]),
                 bias_t=load_blocks('bt', 4, [128, 1]),
                 gvb=load_blocks('gvb', 4, [128, 1]),
                 wq=load_blocks('wq', 4, [128, 64], bf16),
                 wk=load_blocks('wk', 4, [128, 64], bf16),
                 wv=load_blocks('wv', 4, [128, 512], bf16),
                 wc=load_blocks('wc', 4, [128, 104], bf16),
                 debug=debug)
        for nm, p in [('bq', 64), ('bk', 64), ('bc', 104)]:
            C[nm] = cp.tile([p, 1], f32, tag=nm, name=nm)
            nc.sync.dma_start(C[nm][:], t[nm][:])

        ap = est.enter_context(tc.tile_pool(name="actp", bufs=1))
        A = [ap.tile([128, FLAT], bf16, tag=f"A{i}", name=f"A{i}") for i in range(4)]
        B = [ap.tile([128, FLAT], bf16, tag=f"B{i}", name=f"B{i}") for i in range(4)]
        for blk in A + B:
            _zero_act_borders(nc, blk, zeros)
        C['A'], C['B'] = A, B

        # 3 persistent x-strip staging tiles (borders pre-zeroed once)
        xsp = est.enter_context(tc.tile_pool(name="xsp", bufs=1))
        xs_tiles = []
        for i in range(3):
            xs = xsp.tile([128, XS_FLAT], bf16, tag=f"xs{i}", name=f"xs{i}")
            rv = xs[:, 1:1 + 16 * S].rearrange("p (r c) -> p r c", c=S)
            nc.sync.dma_start(xs[:, 0:1], zeros[:, 0:1])
            nc.sync.dma_start(xs[:, XS_FLAT - 2:XS_FLAT], zeros[:, 0:2])
            nc.sync.dma_start(rv[:, :, 64:65], zeros[:, 0:16].unsqueeze(2))
            xs_tiles.append(xs)
        C['xs_tiles'] = xs_tiles

        if debug:
            C['dbgp'] = est.enter_context(tc.tile_pool(name="dbgp", bufs=1))

        for _ in range(n_reps):
            _network(C)


def _zero_act_borders(nc, blk, zeros):
    rv = _rows(blk)
    nc.sync.dma_start(blk[:, 0:1], zeros[:, 0:1])               # lead pad
    nc.sync.dma_start(blk[:, FLAT - 1:FLAT], zeros[:, 0:1])     # slack
    nc.sync.dma_start(rv[:, :, 64:65], zeros[:, 0:NR].unsqueeze(2))
    nc.sync.dma_start(rv[:, 0:1, 0:64], zeros[:, 0:64].unsqueeze(1))
    nc.sync.dma_start(rv[:, 65:67, 0:64],
                      zeros[:, 0:128].rearrange("p (r c) -> p r c", c=64))


def _network(C):
    nc, tc, t = C['nc'], C['tc'], C['t']
    A, B = C['A'], C['B']
    # conva: x strips -> A
    with contextlib.ExitStack() as es:
        wp = es.enter_context(tc.tile_pool(name="wp", bufs=3))
        cps = es.enter_context(tc.tile_pool(name="cps", bufs=1, space="PSUM"))
        xg = _x_strip_getter(C)
        _conv3x3(C, wp, cps, xg, 16, t['wa'], C['bias_a'], A)
    if C['debug']:
        _dump(C, t['o_a'], A)
    # CCA 1: A -> B
    _cca(C, A, B)
    if C['debug']:
        _dump(C, t['o_c1'], B)
    # CCA 2: B -> A
    _cca(C, B, A)
    if C['debug']:
        _dump(C, t['o_c2'], A)
    # convb: A -> B
    with contextlib.ExitStack() as es:
        wp = es.enter_context(tc.tile_pool(name="wpb", bufs=3))
        cps = es.enter_context(tc.tile_pool(name="cpsb", bufs=1, space="PSUM"))
        sg = _act_src_getter(A)
        _conv3x3(C, wp, cps, sg, 4, t['wb'], C['bias_b'], B)
    if C['debug']:
        _dump(C, t['o_b'], B)
    # bott: x strips (16cb) + B (4cb) -> A
    with contextlib.ExitStack() as es:
        wp = es.enter_context(tc.tile_pool(name="wpt", bufs=3))
        cps = es.enter_context(tc.tile_pool(name="cpst", bufs=1, space="PSUM"))
        xg = _x_strip_getter(C)
        sg = _act_src_getter(B)

        def src_get(g, cb):
            return xg(g, cb) if cb < 16 else sg(g, cb - 16)

        _conv3x3(C, wp, cps, src_get, 20, t['wt'], C['bias_t'], A)
    if C['debug']:
        _dump(C, t['o_t'], A)
    # cls: A -> y
    with contextlib.ExitStack() as es:
        cop = es.enter_context(tc.tile_pool(name="cop", bufs=1))
        cpp = es.enter_context(tc.tile_pool(name="cpp", bufs=2, space="PSUM"))
        out_sb = cop.tile([104, 64, 64], f32)
        for r0, nr in CLS_STRIPS:
            n = nr * S + 1
            ps = cpp.tile([104, n], f32, tag="clsps")
            for cb in range(4):
                rhs = A[cb][:, IMG0 + r0 * S:IMG0 + r0 * S + n]
                nc.tensor.matmul(ps[:], C['wc'][cb][:], rhs,
                                 start=(cb == 0), stop=(cb == 3))
            pv = ps[:, 0:nr * S].rearrange("p (r c) -> p r c", c=S)[:, :, 0:64]
            nc.scalar.activation(out_sb[:, r0:r0 + nr, :], pv, AF.Identity,
                                 bias=C['bc'][:], scale=1.0)
        nc.sync.dma_start(C['y'][:], out_sb[:])


def _x_strip_getter(C):
    """Rotating x-strip loader: 1 big row-load per (g, cb) + edge zeroing."""
    nc, zeros, t = C['nc'], C['zeros'], C['t']
    tiles = C['xs_tiles']
    state = {'i': 0}
    cache = {}

    def get(g, cb):
        key = (g, cb)
        if key in cache:
            return cache[key]
        xs = tiles[state['i'] % 3]
        state['i'] += 1
        rv = xs[:, 1:1 + 16 * S].rearrange("p (r c) -> p r c", c=S)
        r0g = GROUP_R0[g]
        lo = max(0, r0g - 1)
        hi = min(64, r0g + 15)
        l0, l1 = lo - (r0g - 1), lo - (r0g - 1) + hi - lo
        eng = nc.sync if V_XSYNC else nc.scalar
        if l0 > 0:
            eng.dma_start(rv[:, 0:l0, 0:64],
                          zeros[:, 0:l0 * 64].rearrange("p (r c) -> p r c", c=64))
        if l1 < 16:
            eng.dma_start(rv[:, l1:16, 0:64],
                          zeros[:, 0:(16 - l1) * 64].rearrange("p (r c) -> p r c", c=64))
        eng.dma_start(rv[:, l0:l1, 0:64], t['x'][cb][:, lo:hi, :])
        res = (xs, lambda r0, _g=r0g: r0 - _g + 1)
        cache[key] = res
        # only keep entries for the current group alive in the rotation
        for k in list(cache):
            if k[0] != g:
                del cache[k]
        return res

    return get


def _act_src_getter(blocks):
    def get(g, cb):
        return (blocks[cb], lambda r0: r0 + 1)
    return get


def _conv3x3(C, wp, cps, src_getter, n_cb, w_dram, bias_sb, dst_set):
    nc = C['nc']
    for g, strips in enumerate(GROUPS):
        psums = {}
        for si, (r0, nr) in enumerate(strips):
            for co in range(4):
                psums[(si, co)] = cps.tile([128, nr * S + 1], f32,
                                           tag=f"c{si}{co}", name=f"c{si}{co}")
        for cb in range(n_cb):
            wtl = wp.tile([128, 9, 512], bf16, tag="w")
            nc.sync.dma_start(wtl[:], w_dram[cb])
            sflat, base_row = src_getter(g, cb)
            for tap in range(9):
                dy, dx = tap // 3 - 1, tap % 3 - 1
                for co in range(4):
                    for si, (r0, nr) in enumerate(strips):
                        n = nr * S + 1
                        off = 1 + (base_row(r0) + dy) * S + dx
                        nc.tensor.matmul(
                            psums[(si, co)][:],
                            wtl[:, tap, co * 128:(co + 1) * 128],
                            sflat[:, off:off + n],
                            start=(cb == 0 and tap == 0),
                            stop=(cb == n_cb - 1 and tap == 8))
        for si, (r0, nr) in enumerate(strips):
            for co in range(4):
                ps = psums[(si, co)]
                pv = ps[:, 0:nr * S].rearrange("p (r c) -> p r c", c=S)[:, :, 0:64]
                dst = _rows(dst_set[co])[:, 1 + r0:1 + r0 + nr, 0:64]
                nc.scalar.activation(dst, pv, AF.Relu, bias=bias_sb[co], scale=1.0)


def _cca(C, SRC, DST):
    """One criss-cross attention: DST = gamma*(outh+outw+v_b) + SRC."""
    nc, tc = C['nc'], C['tc']
    gamma, ident, maskd = C['gamma'], C['ident'], C['maskd']
    edt = bf16 if V_BF16T else f32
    eident = ident if V_BF16T else C['identf']
    with contextlib.ExitStack() as es:
        qkp = es.enter_context(tc.tile_pool(name="qkp", bufs=1))
        atp = es.enter_context(tc.tile_pool(name="atp", bufs=1))
        smp = es.enter_context(tc.tile_pool(name="smp", bufs=1))
        esA = es.enter_context(contextlib.ExitStack())
        psQ = esA.enter_context(tc.tile_pool(name="psQ", bufs=2, space="PSUM"))
        psE = esA.enter_context(tc.tile_pool(name="psE", bufs=2, space="PSUM"))

        q_sb = qkp.tile([64, 64, 65], bf16, tag="q")
        k_sb = qkp.tile([64, 64, 65], bf16, tag="k")
        # ---- q/k 1x1 convs
        for dst_sb, wgt, bias in [(q_sb, C['wq'], C['bq']),
                                  (k_sb, C['wk'], C['bk'])]:
            dflat = dst_sb[:].rearrange("p r c -> p (r c)")
            for off, n in QK_TILES:
                ps = psQ.tile([64, 512], f32, tag="qkps")
                for cb in range(4):
                    rhs = SRC[cb][:, IMG0 + off:IMG0 + off + n]
                    nc.tensor.matmul(ps[:, 0:n], wgt[cb][:], rhs,
                                     start=(cb == 0), stop=(cb == 3))
                nc.scalar.activation(dflat[:, off:off + n], ps[:, 0:n],
                                     AF.Identity, bias=bias[:], scale=1.0)
        # ---- energies + exp + per-slice sums
        # EH[h, w, j] (attention over height, per column w), diag-masked.
        # EW[w, h, j] (attention over width, per row h).
        EH = smp.tile([64, 64, 64], edt, tag="EH")
        EW = smp.tile([64, 64, 64], edt, tag="EW")
        ZH = smp.tile([64, 64], f32, tag="ZH")    # [h, w] sum_j exp(eh)
        ZW = smp.tile([64, 64], f32, tag="ZW")    # [w, h] sum_j exp(ew)
        for c0 in range(0, 64, 16):
            pe = psE.tile([64, 16, 64], f32, tag="pe")
            for wi in range(16):
                w = c0 + wi
                nc.tensor.matmul(pe[:, wi, :], q_sb[:, :, w], k_sb[:, :, w],
                                 start=True, stop=False)
                nc.tensor.matmul(pe[:, wi, :], ident[:], maskd[:],
                                 start=False, stop=True)
            nc.scalar.activation(EH[:, c0:c0 + 16, :], pe[:], AF.Exp)
            nc.vector.tensor_reduce(ZH[:, c0:c0 + 16], EH[:, c0:c0 + 16, :],
                                    mybir.AxisListType.X, ALU.add)
        for c0 in range(0, 64, 16):
            pe = psE.tile([64, 16, 64], f32, tag="pe")
            for hi in range(16):
                h = c0 + hi
                nc.tensor.matmul(pe[:, hi, :], q_sb[:, h, 0:64],
                                 k_sb[:, h, 0:64], start=True, stop=True)
            nc.scalar.activation(EW[:, c0:c0 + 16, :], pe[:], AF.Exp)
            nc.vector.tensor_reduce(ZW[:, c0:c0 + 16], EW[:, c0:c0 + 16, :],
                                    mybir.AxisListType.X, ALU.add)
        esA.close()
        esB = es.enter_context(contextlib.ExitStack())
        psZ = esB.enter_context(tc.tile_pool(name="psZ", bufs=1, space="PSUM"))
        psT = esB.enter_context(tc.tile_pool(name="psT", bufs=3, space="PSUM"))
        # ---- joint normalizers: R[h, w] = 1/(ZH + ZW^T), R2 = R^T
        ZWT = psZ.tile([64, 64], f32, tag="zt")
        nc.tensor.transpose(ZWT[:], ZW[:], C['identf'][:])
        R = smp.tile([64, 64], f32, tag="R")
        nc.vector.tensor_tensor(R[:], ZH[:], ZWT[:], ALU.add)
        nc.vector.reciprocal(R[:], R[:])
        RT = psZ.tile([64, 64], f32, tag="zt2")
        nc.tensor.transpose(RT[:], R[:], C['identf'][:])
        R2 = smp.tile([64, 64], f32, tag="R2")
        nc.vector.tensor_copy(R2[:], RT[:])
        # ---- normalize in place
        if V_BCAST:
            for c0 in range(0, 64, 16):
                nc.vector.tensor_tensor(
                    EH[:, c0:c0 + 16, :], EH[:, c0:c0 + 16, :],
                    R[:, c0:c0 + 16].unsqueeze(2).broadcast_to([64, 16, 64]),
                    ALU.mult)
                nc.vector.tensor_tensor(
                    EW[:, c0:c0 + 16, :], EW[:, c0:c0 + 16, :],
                    R2[:, c0:c0 + 16].unsqueeze(2).broadcast_to([64, 16, 64]),
                    ALU.mult)
        else:
            for u in range(64):
                nc.vector.tensor_scalar_mul(EH[:, u, :], EH[:, u, :],
                                            R[:, u:u + 1])
                nc.vector.tensor_scalar_mul(EW[:, u, :], EW[:, u, :],
                                            R2[:, u:u + 1])
        # ---- transposes
        if V_PAIR:
            # ATh[128=(par,j), 32, 64h], ATw[128, 32, 64w]
            ATh = atp.tile([128, 32, 64], bf16, tag="ATh")
            ATw = atp.tile([128, 32, 64], bf16, tag="ATw")
            for E, AT in [(EH, ATh), (EW, ATw)]:
                for p in range(32):
                    pst = psT.tile([128, 64], edt, tag="pt")
                    nc.tensor.transpose(
                        pst[:],
                        E[:, 2 * p:2 * p + 2, :].rearrange("p a b -> p (a b)"),
                        eident[:])
                    nc.scalar.activation(AT[:, p, :], pst[:], AF.Copy)
        else:
            ATh = atp.tile([64, 64, 64], bf16, tag="ATh")
            ATw = atp.tile([64, 64, 64], bf16, tag="ATw")
            for E, AT in [(EH, ATh), (EW, ATw)]:
                for u in range(64):
                    pst = psT.tile([64, 64], edt, tag="pt")
                    nc.tensor.transpose(pst[:], E[:, u, :], eident[:])
                    nc.scalar.activation(AT[:, u, :], pst[:], AF.Copy)
        esB.close()
        # ---- apply: w-phase (out_h) then h-phase (out_w)
        with contextlib.ExitStack() as esC:
            vtp = esC.enter_context(tc.tile_pool(name="vtp", bufs=6))
            psV = esC.enter_context(tc.tile_pool(name="psV", bufs=3, space="PSUM"))
            psO = esC.enter_context(tc.tile_pool(name="psO", bufs=4, space="PSUM"))
            for phase in range(2):  # 0: w-phase (per-column), 1: h-phase (per-row)
                AT = ATh if phase == 0 else ATw
                for c4 in range(16):  # chunks of 4 columns/rows
                    vts = []
                    if V_PAIR:
                        for pr in range(2):
                            u0 = c4 * 4 + pr * 2
                            pv = psV.tile([128, 512], f32, tag="pv")
                            for cb in range(4):
                                rv = _rows(SRC[cb])
                                # stationary operand needs one contiguous
                                # free dim: stage the (pair, 64) slab first
                                vstg = vtp.tile([128, 2, 64], bf16, tag="vstg")
                                if phase == 0:
                                    nc.vector.tensor_copy(
                                        vstg[:],
                                        rv[:, 1:65, u0:u0 + 2].rearrange(
                                            "p r w -> p w r"))
                                else:
                                    nc.vector.tensor_copy(
                                        vstg[:], rv[:, 1 + u0:3 + u0, 0:64])
                                nc.tensor.matmul(
                                    pv[:], vstg[:].rearrange("p a b -> p (a b)"),
                                    C['wv'][cb][:],
                                    start=(cb == 0), stop=(cb == 3))
                            vt = vtp.tile([128, 512], bf16, tag="vt")
                            nc.scalar.activation(vt[:], pv[:], AF.Copy)
                            vts.append(vt)
                    else:
                        for i in range(4):
                            u = c4 * 4 + i
                            pv = psV.tile([64, 512], f32, tag="pv")
                            for cb in range(4):
                                rv = _rows(SRC[cb])
                                lhsT = (rv[:, 1:65, u] if phase == 0
                                        else rv[:, 1 + u, 0:64])
                                nc.tensor.matmul(pv[:], lhsT, C['wv'][cb][:],
                                                 start=(cb == 0), stop=(cb == 3))
                            vt = vtp.tile([64, 512], bf16, tag="vt")
                            nc.scalar.activation(vt[:], pv[:], AF.Copy)
                            vts.append(vt)
                    for cbo in range(4):
                        po = psO.tile([128, 4, 64], f32, tag="po")
                        for i in range(4):
                            if V_PAIR:
                                pr, par = i // 2, (i % 2) * 64
                                lhsT = vts[pr][par:par + 64,
                                               cbo * 128:(cbo + 1) * 128]
                                rhs = AT[par:par + 64, c4 * 2 + pr, :]
                            else:
                                lhsT = vts[i][:, cbo * 128:(cbo + 1) * 128]
                                rhs = AT[:, c4 * 4 + i, :]
                            nc.tensor.matmul(po[:, i, :], lhsT, rhs,
                                             start=True, stop=True)
                        rvD = _rows(DST[cbo])
                        rvS = _rows(SRC[cbo])
                        if phase == 0:
                            o_sl = rvD[:, 1:65, c4 * 4:c4 * 4 + 4].rearrange(
                                "p h w -> p w h")
                            i_sl = rvS[:, 1:65, c4 * 4:c4 * 4 + 4].rearrange(
                                "p h w -> p w h")
                            nc.vector.scalar_tensor_tensor(
                                o_sl, po[:], gamma, i_sl, ALU.mult, ALU.add)
                        else:
                            o_sl = rvD[:, 1 + c4 * 4:5 + c4 * 4, 0:64]
                            nc.vector.scalar_tensor_tensor(
                                o_sl, po[:], gamma, o_sl, ALU.mult, ALU.add)
        # ---- + gamma * v_b (joint softmax sums to 1 across both branches)
        for cbo in range(4):
            o_in = _rows(DST[cbo])[:, 1:65, 0:64]
            nc.vector.tensor_scalar_add(o_in, o_in, C['gvb'][cbo][:])


_BUILD_CACHE = {}


def _get_nc(gamma):
    key = round(float(gamma), 12)
    if key not in _BUILD_CACHE:
        _BUILD_CACHE[key] = build(gamma, n_reps=1)
    return _BUILD_CACHE[key]


def kernel(**inputs):
    from concourse.bass_utils import run_bass_kernel_spmd
    inputs_np = {k: np.asarray(v) for k, v in inputs.items()}
    dev, gamma = host_prep(inputs_np)
    nc = _get_nc(gamma)
    xbf = inputs_np['x'].astype(ml_dtypes.bfloat16)
    in_maps = []
    for core in range(8):
        m = dict(dev)
        m['x'] = np.ascontiguousarray(xbf[core].reshape(*X_DEV_SHAPE))
        in_maps.append(m)
    res = run_bass_kernel_spmd(nc, in_maps, core_ids=list(range(8)))
    out = np.stack([r['y'].reshape(104, 64, 64) for r in res.results])
    return out.astype(np.float32)


# revision 14
# speedup vs baseline: 1.2015x; 1.1966x over previous
"""Trainium2 Bass kernel for CCHead (criss-cross attention head).

Self-contained: kernel(**inputs) takes the full unsharded inputs
(x[8, 2048, 64, 64] + weights), shards batch across 8 NeuronCores
(1 image per core, all params replicated), and returns the full
output [8, 104, 64, 64] float32.

v2: all matmul operands bf16 (f32r streams 4-byte operands at 2
cycles/row on HW; bf16 runs 1 cycle/row), activations ping-pong
between two SBUF-resident padded buffer sets (no DRAM round trips
between stages), CCA restructured (paired VT/transpose matmuls,
single-pass softmax ops).
"""
import contextlib
import os

import numpy as np
import ml_dtypes

import concourse.bass as bass
import concourse.tile as tile
from concourse import bacc, mybir

V_BCAST = os.environ.get('CC_BCAST', '1') == '1'   # stride-0 broadcast normalize
# base-64 tile_position matmuls (paired transposes/VT) crash the device
# (NRT_EXEC_UNIT_UNRECOVERABLE); keep the per-column path.
V_PAIR = os.environ.get('CC_PAIR', '0') == '1'
V_BF16T = os.environ.get('CC_BF16T', '1') == '1'   # bf16 psum transposes (EH/EW bf16)
V_XSYNC = os.environ.get('CC_XSYNC', '1') == '1'   # x strips on sync HWDGE ring


f32 = mybir.dt.float32
bf16 = mybir.dt.bfloat16
AF = mybir.ActivationFunctionType
ALU = mybir.AluOpType

X_DEV_SHAPE = (16, 128, 64, 64)

S = 65
NR = 67
FLAT = NR * S + 2          # 4357
IMG0 = 1 + S               # flat offset of image row 0, col 0 = 66
STRIPS = [(r, 7) for r in range(0, 56, 7)] + [(56, 4), (60, 4)]
GROUPS = [STRIPS[0:2], STRIPS[2:4], STRIPS[4:6], STRIPS[6:8], STRIPS[8:10]]
GROUP_R0 = [0, 14, 28, 42, 56]
XS_FLAT = 16 * S + 3       # 1 lead pad + 16 rows * 65 + 2 slack
QK_TILES = [(i * 512, 512) for i in range(8)] + [(4096, 64)]
CLS_STRIPS = [(r, 7) for r in range(0, 56, 7)] + [(56, 4), (60, 4)]


def host_prep(inputs):
    f = np.float32
    bf = ml_dtypes.bfloat16

    def fold(w, g, b, m, v):
        s = (g / np.sqrt(v + 1e-5)).astype(f)
        return (w * s[:, None, None, None]).astype(f), (b - m * s).astype(f)

    def wt_dev(w):  # [co, ci, 3, 3] -> [nci, 128, 9, co] bf16
        co, ci = w.shape[:2]
        return np.ascontiguousarray(
            w.reshape(co, ci, 9).transpose(1, 2, 0).reshape(
                ci // 128, 128, 9, co).astype(bf))

    def t1x1(w):  # [co, ci, 1, 1] -> [nci, 128, co] bf16
        co, ci = w.shape[:2]
        return np.ascontiguousarray(
            w.reshape(co, ci).T.reshape(ci // 128, 128, co).astype(bf))

    wa, ba = fold(inputs['conva_w'], inputs['conva_g'], inputs['conva_b'],
                  inputs['conva_m'], inputs['conva_v'])
    wb, bb = fold(inputs['convb_w'], inputs['convb_g'], inputs['convb_b'],
                  inputs['convb_m'], inputs['convb_v'])
    wt, bt = fold(inputs['bott_w'], inputs['bott_g'], inputs['bott_b'],
                  inputs['bott_m'], inputs['bott_v'])
    gamma = float(np.asarray(inputs['cc_gamma']).reshape(-1)[0])
    maskd = np.zeros((64, 64), f)
    np.fill_diagonal(maskd, -1e30)
    dev = {
        'wa': wt_dev(wa), 'ba': ba.reshape(4, 128, 1),
        'wb': wt_dev(wb), 'bb': bb.reshape(4, 128, 1),
        'wt': wt_dev(wt), 'bt': bt.reshape(4, 128, 1),
        'wc': t1x1(inputs['cls_w']), 'bc': inputs['cls_b'].astype(f).reshape(104, 1),
        'wq': t1x1(inputs['q_w']), 'bq': inputs['q_b'].astype(f).reshape(64, 1),
        'wk': t1x1(inputs['k_w']), 'bk': inputs['k_b'].astype(f).reshape(64, 1),
        'wv': t1x1(inputs['v_w']),
        'gvb': (gamma * inputs['v_b']).astype(f).reshape(4, 128, 1),
        'maskd': maskd.astype(bf),
        'zerosl': np.zeros((128, 1056), bf),
        'ident': np.eye(64, dtype=bf),
        'identf': np.eye(64, dtype=f),
    }
    return dev, gamma


INPUT_SPECS = [
    ('wa', [16, 128, 9, 512], bf16), ('ba', [4, 128, 1], f32),
    ('wb', [4, 128, 9, 512], bf16), ('bb', [4, 128, 1], f32),
    ('wt', [20, 128, 9, 512], bf16), ('bt', [4, 128, 1], f32),
    ('wc', [4, 128, 104], bf16), ('bc', [104, 1], f32),
    ('wq', [4, 128, 64], bf16), ('bq', [64, 1], f32),
    ('wk', [4, 128, 64], bf16), ('bk', [64, 1], f32),
    ('wv', [4, 128, 512], bf16),
    ('gvb', [4, 128, 1], f32),
    ('maskd', [64, 64], bf16),
    ('zerosl', [128, 1056], bf16),
    ('ident', [64, 64], bf16),
    ('identf', [64, 64], f32),
]


def build(gamma, n_reps=1, debug=False):
    nc = bacc.Bacc("TRN2", num_devices=8)
    t = {'x': nc.dram_tensor("x", list(X_DEV_SHAPE), bf16, kind="ExternalInput")}
    for nm, shape, dt in INPUT_SPECS:
        t[nm] = nc.dram_tensor(nm, shape, dt, kind="ExternalInput")
    y = nc.dram_tensor("y", [104, 64, 64], f32, kind="ExternalOutput")
    if debug:
        for nm in ['o_a', 'o_c1', 'o_c2', 'o_b', 'o_t']:
            t[nm] = nc.dram_tensor(nm, [4, 128, 64, 64], f32,
                                   kind="ExternalOutput")
    with tile.TileContext(nc) as tc:
        _build_body(tc, t, y, gamma, n_reps, debug)
    nc.compile()
    return nc


def _rows(flat_tile):
    """[128, FLAT] -> padded row view [128, 67, 65] (skips lead pad elem)."""
    return flat_tile[:, 1:1 + NR * S].rearrange("p (r c) -> p r c", c=S)


def _dump(C, dram4, blocks):
    nc = C['nc']
    for cb in range(4):
        stg = C['dbgp'].tile([128, 64, 64], f32, tag="dbg")
        nc.vector.tensor_copy(stg[:], _rows(blocks[cb])[:, 1:65, 0:64])
        nc.sync.dma_start(dram4[cb], stg[:])


def _build_body(tc, t, y, gamma, n_reps, debug):
    nc = tc.nc
    with contextlib.ExitStack() as est:
        cp = est.enter_context(tc.tile_pool(name="const", bufs=1))
        zeros = cp.tile([128, 1056], bf16)
        nc.sync.dma_start(zeros[:], t['zerosl'][:])
        ident = cp.tile([64, 64], bf16)
        nc.sync.dma_start(ident[:], t['ident'][:])
        maskd = cp.tile([64, 64], bf16)
        nc.sync.dma_start(maskd[:], t['maskd'][:])
        identf = cp.tile([64, 64], f32)
        nc.sync.dma_start(identf[:], t['identf'][:])

        def load_blocks(nm, n, shape, dt=f32):
            out = []
            for i in range(n):
                tl = cp.tile(shape, dt, tag=f"{nm}{i}", name=f"{nm}{i}")
                nc.sync.dma_start(tl[:], t[nm][i])
                out.append(tl)
            return out

        C = dict(nc=nc, tc=tc, t=t, y=y, gamma=gamma, zeros=zeros, ident=ident,
                 identf=identf,
                 maskd=maskd,
                 bias_a=load_blocks('ba', 4, [128, 1]),
                 bias_b=load_blocks('bb', 4, [128, 1]),
                 bias_t=load_blocks('bt', 4, [128, 1]),
                 gvb=load_blocks('gvb', 4, [128, 1]),
                 wq=load_blocks('wq', 4, [128, 64], bf16),
                 wk=load_blocks('wk', 4, [128, 64], bf16),
                 wv=load_blocks('wv', 4, [128, 512], bf16),
                 wc=load_blocks('wc', 4, [128, 104], bf16),
                 debug=debug)
        for nm, p in [('bq', 64), ('bk', 64), ('bc', 104)]:
            C[nm] = cp.tile([p, 1], f32, tag=nm, name=nm)
            nc.sync.dma_start(C[nm][:], t[nm][:])

        ap = est.enter_context(tc.tile_pool(name="actp", bufs=1))
        A = [ap.tile([128, FLAT], bf16, tag=f"A{i}", name=f"A{i}") for i in range(4)]
        B = [ap.tile([128, FLAT], bf16, tag=f"B{i}", name=f"B{i}") for i in range(4)]
        for blk in A + B:
            _zero_act_borders(nc, blk, zeros)
        C['A'], C['B'] = A, B

        # 3 persistent x-strip staging tiles (borders pre-zeroed once)
        xsp = est.enter_context(tc.tile_pool(name="xsp", bufs=1))
        xs_tiles = []
        for i in range(3):
            xs = xsp.tile([128, XS_FLAT], bf16, tag=f"xs{i}", name=f"xs{i}")
            rv = xs[:, 1:1 + 16 * S].rearrange("p (r c) -> p r c", c=S)
            nc.sync.dma_start(xs[:, 0:1], zeros[:, 0:1])
            nc.sync.dma_start(xs[:, XS_FLAT - 2:XS_FLAT], zeros[:, 0:2])
            nc.sync.dma_start(rv[:, :, 64:65], zeros[:, 0:16].unsqueeze(2))
            xs_tiles.append(xs)
        C['xs_tiles'] = xs_tiles

        if debug:
            C['dbgp'] = est.enter_context(tc.tile_pool(name="dbgp", bufs=1))

        for _ in range(n_reps):
            _network(C)


def _zero_act_borders(nc, blk, zeros):
    rv = _rows(blk)
    nc.sync.dma_start(blk[:, 0:1], zeros[:, 0:1])               # lead pad
    nc.sync.dma_start(blk[:, FLAT - 1:FLAT], zeros[:, 0:1])     # slack
    nc.sync.dma_start(rv[:, :, 64:65], zeros[:, 0:NR].unsqueeze(2))
    nc.sync.dma_start(rv[:, 0:1, 0:64], zeros[:, 0:64].unsqueeze(1))
    nc.sync.dma_start(rv[:, 65:67, 0:64],
                      zeros[:, 0:128].rearrange("p (r c) -> p r c", c=64))


def _network(C):
    nc, tc, t = C['nc'], C['tc'], C['t']
    A, B = C['A'], C['B']
    # conva: x strips -> A
    with contextlib.ExitStack() as es:
        wp = es.enter_context(tc.tile_pool(name="wp", bufs=3))
        cps = es.enter_context(tc.tile_pool(name="cps", bufs=1, space="PSUM"))
        xg = _x_strip_getter(C)
        _conv3x3(C, wp, cps, xg, 16, t['wa'], C['bias_a'], A)
    if C['debug']:
        _dump(C, t['o_a'], A)
    # CCA 1: A -> B
    _cca(C, A, B)
    if C['debug']:
        _dump(C, t['o_c1'], B)
    # CCA 2: B -> A
    _cca(C, B, A)
    if C['debug']:
        _dump(C, t['o_c2'], A)
    # convb: A -> B
    with contextlib.ExitStack() as es:
        wp = es.enter_context(tc.tile_pool(name="wpb", bufs=3))
        cps = es.enter_context(tc.tile_pool(name="cpsb", bufs=1, space="PSUM"))
        sg = _act_src_getter(A)
        _conv3x3(C, wp, cps, sg, 4, t['wb'], C['bias_b'], B)
    if C['debug']:
        _dump(C, t['o_b'], B)
    # bott: x strips (16cb) + B (4cb) -> A
    with contextlib.ExitStack() as es:
        wp = es.enter_context(tc.tile_pool(name="wpt", bufs=3))
        cps = es.enter_context(tc.tile_pool(name="cpst", bufs=1, space="PSUM"))
        xg = _x_strip_getter(C)
        sg = _act_src_getter(B)

        def src_get(g, cb):
            return xg(g, cb) if cb < 16 else sg(g, cb - 16)

        _conv3x3(C, wp, cps, src_get, 20, t['wt'], C['bias_t'], A)
    if C['debug']:
        _dump(C, t['o_t'], A)
    # cls: A -> y
    with contextlib.ExitStack() as es:
        cop = es.enter_context(tc.tile_pool(name="cop", bufs=1))
        cpp = es.enter_context(tc.tile_pool(name="cpp", bufs=2, space="PSUM"))
        out_sb = cop.tile([104, 64, 64], f32)
        for r0, nr in CLS_STRIPS:
            n = nr * S + 1
            ps = cpp.tile([104, n], f32, tag="clsps")
            for cb in range(4):
                rhs = A[cb][:, IMG0 + r0 * S:IMG0 + r0 * S + n]
                nc.tensor.matmul(ps[:], C['wc'][cb][:], rhs,
                                 start=(cb == 0), stop=(cb == 3))
            pv = ps[:, 0:nr * S].rearrange("p (r c) -> p r c", c=S)[:, :, 0:64]
            nc.scalar.activation(out_sb[:, r0:r0 + nr, :], pv, AF.Identity,
                                 bias=C['bc'][:], scale=1.0)
        nc.sync.dma_start(C['y'][:], out_sb[:])


def _x_strip_getter(C):
    """Rotating x-strip loader: 1 big row-load per (g, cb) + edge zeroing."""
    nc, zeros, t = C['nc'], C['zeros'], C['t']
    tiles = C['xs_tiles']
    state = {'i': 0}
    cache = {}

    def get(g, cb):
        key = (g, cb)
        if key in cache:
            return cache[key]
        xs = tiles[state['i'] % 3]
        state['i'] += 1
        rv = xs[:, 1:1 + 16 * S].rearrange("p (r c) -> p r c", c=S)
        r0g = GROUP_R0[g]
        lo = max(0, r0g - 1)
        hi = min(64, r0g + 15)
        l0, l1 = lo - (r0g - 1), lo - (r0g - 1) + hi - lo
        eng = nc.sync if V_XSYNC else nc.scalar
        if l0 > 0:
            eng.dma_start(rv[:, 0:l0, 0:64],
                          zeros[:, 0:l0 * 64].rearrange("p (r c) -> p r c", c=64))
        if l1 < 16:
            eng.dma_start(rv[:, l1:16, 0:64],
                          zeros[:, 0:(16 - l1) * 64].rearrange("p (r c) -> p r c", c=64))
        eng.dma_start(rv[:, l0:l1, 0:64], t['x'][cb][:, lo:hi, :])
        res = (xs, lambda r0, _g=r0g: r0 - _g + 1)
        cache[key] = res
        # only keep entries for the current group alive in the rotation
        for k in list(cache):
            if k[0] != g:
                del cache[k]
        return res

    return get


def _act_src_getter(blocks):
    def get(g, cb):
        return (blocks[cb], lambda r0: r0 + 1)
    return get


def _conv3x3(C, wp, cps, src_getter, n_cb, w_dram, bias_sb, dst_set):
    nc = C['nc']
    for g, strips in enumerate(GROUPS):
        psums = {}
        for si, (r0, nr) in enumerate(strips):
            for co in range(4):
                psums[(si, co)] = cps.tile([128, nr * S + 1], f32,
                                           tag=f"c{si}{co}", name=f"c{si}{co}")
        for cb in range(n_cb):
            wtl = wp.tile([128, 9, 512], bf16, tag="w")
            nc.sync.dma_start(wtl[:], w_dram[cb])
            sflat, base_row = src_getter(g, cb)
            for tap in range(9):
                dy, dx = tap // 3 - 1, tap % 3 - 1
                for co in range(4):
                    for si, (r0, nr) in enumerate(strips):
                        n = nr * S + 1
                        off = 1 + (base_row(r0) + dy) * S + dx
                        nc.tensor.matmul(
                            psums[(si, co)][:],
                            wtl[:, tap, co * 128:(co + 1) * 128],
                            sflat[:, off:off + n],
                            start=(cb == 0 and tap == 0),
                            stop=(cb == n_cb - 1 and tap == 8))
        for si, (r0, nr) in enumerate(strips):
            for co in range(4):
                ps = psums[(si, co)]
                pv = ps[:, 0:nr * S].rearrange("p (r c) -> p r c", c=S)[:, :, 0:64]
                dst = _rows(dst_set[co])[:, 1 + r0:1 + r0 + nr, 0:64]
                nc.scalar.activation(dst, pv, AF.Relu, bias=bias_sb[co], scale=1.0)


def _cca(C, SRC, DST):
    """One criss-cross attention: DST = gamma*(outh+outw+v_b) + SRC."""
    nc, tc = C['nc'], C['tc']
    gamma, ident, maskd = C['gamma'], C['ident'], C['maskd']
    edt = bf16 if V_BF16T else f32
    eident = ident if V_BF16T else C['identf']
    with contextlib.ExitStack() as es:
        qkp = es.enter_context(tc.tile_pool(name="qkp", bufs=1))
        atp = es.enter_context(tc.tile_pool(name="atp", bufs=1))
        smp = es.enter_context(tc.tile_pool(name="smp", bufs=1))
        esA = es.enter_context(contextlib.ExitStack())
        psQ = esA.enter_context(tc.tile_pool(name="psQ", bufs=2, space="PSUM"))
        psE = esA.enter_context(tc.tile_pool(name="psE", bufs=2, space="PSUM"))

        q_sb = qkp.tile([64, 64, 65], bf16, tag="q")
        k_sb = qkp.tile([64, 64, 65], bf16, tag="k")
        # ---- q/k 1x1 convs
        for dst_sb, wgt, bias in [(q_sb, C['wq'], C['bq']),
                                  (k_sb, C['wk'], C['bk'])]:
            dflat = dst_sb[:].rearrange("p r c -> p (r c)")
            for off, n in QK_TILES:
                ps = psQ.tile([64, 512], f32, tag="qkps")
                for cb in range(4):
                    rhs = SRC[cb][:, IMG0 + off:IMG0 + off + n]
                    nc.tensor.matmul(ps[:, 0:n], wgt[cb][:], rhs,
                                     start=(cb == 0), stop=(cb == 3))
                nc.scalar.activation(dflat[:, off:off + n], ps[:, 0:n],
                                     AF.Identity, bias=bias[:], scale=1.0)
        # ---- energies + exp + per-slice sums
        # EH[h, w, j] (attention over height, per column w), diag-masked.
        # EW[w, h, j] (attention over width, per row h).
        EH = smp.tile([64, 64, 64], edt, tag="EH")
        EW = smp.tile([64, 64, 64], edt, tag="EW")
        ZH = smp.tile([64, 64], f32, tag="ZH")    # [h, w] sum_j exp(eh)
        ZW = smp.tile([64, 64], f32, tag="ZW")    # [w, h] sum_j exp(ew)
        for c0 in range(0, 64, 16):
            pe = psE.tile([64, 16, 64], f32, tag="pe")
            for wi in range(16):
                w = c0 + wi
                nc.tensor.matmul(pe[:, wi, :], q_sb[:, :, w], k_sb[:, :, w],
                                 start=True, stop=False)
                nc.tensor.matmul(pe[:, wi, :], ident[:], maskd[:],
                                 start=False, stop=True)
            nc.scalar.activation(EH[:, c0:c0 + 16, :], pe[:], AF.Exp)
            nc.vector.tensor_reduce(ZH[:, c0:c0 + 16], EH[:, c0:c0 + 16, :],
                                    mybir.AxisListType.X, ALU.add)
        for c0 in range(0, 64, 16):
            pe = psE.tile([64, 16, 64], f32, tag="pe")
            for hi in range(16):
                h = c0 + hi
                nc.tensor.matmul(pe[:, hi, :], q_sb[:, h, 0:64],
                                 k_sb[:, h, 0:64], start=True, stop=True)
            nc.scalar.activation(EW[:, c0:c0 + 16, :], pe[:], AF.Exp)
            nc.vector.tensor_reduce(ZW[:, c0:c0 + 16], EW[:, c0:c0 + 16, :],
                                    mybir.AxisListType.X, ALU.add)
        esA.close()
        esB = es.enter_context(contextlib.ExitStack())
        psZ = esB.enter_context(tc.tile_pool(name="psZ", bufs=1, space="PSUM"))
        psT = esB.enter_context(tc.tile_pool(name="psT", bufs=3, space="PSUM"))
        # ---- joint normalizers: R[h, w] = 1/(ZH + ZW^T), R2 = R^T
        ZWT = psZ.tile([64, 64], f32, tag="zt")
        nc.tensor.transpose(ZWT[:], ZW[:], C['identf'][:])
        R = smp.tile([64, 64], f32, tag="R")
        nc.vector.tensor_tensor(R[:], ZH[:], ZWT[:], ALU.add)
        nc.vector.reciprocal(R[:], R[:])
        RT = psZ.tile([64, 64], f32, tag="zt2")
        nc.tensor.transpose(RT[:], R[:], C['identf'][:])
        R2 = smp.tile([64, 64], f32, tag="R2")
        nc.vector.tensor_copy(R2[:], RT[:])
        # ---- normalize in place
        if V_BCAST:
            for c0 in range(0, 64, 16):
                nc.vector.tensor_tensor(
                    EH[:, c0:c0 + 16, :], EH[:, c0:c0 + 16, :],
                    R[:, c0:c0 + 16].unsqueeze(2).broadcast_to([64, 16, 64]),
                    ALU.mult)
                nc.vector.tensor_tensor(
                    EW[:, c0:c0 + 16, :], EW[:, c0:c0 + 16, :],
                    R2[:, c0:c0 + 16].unsqueeze(2).broadcast_to([64, 16, 64]),
                    ALU.mult)
        else:
            for u in range(64):
                nc.vector.tensor_scalar_mul(EH[:, u, :], EH[:, u, :],
                                            R[:, u:u + 1])
                nc.vector.tensor_scalar_mul(EW[:, u, :], EW[:, u, :],
                                            R2[:, u:u + 1])
        # ---- transposes
        if V_PAIR:
            # ATh[128=(par,j), 32, 64h], ATw[128, 32, 64w]
            ATh = atp.tile([128, 32, 64], bf16, tag="ATh")
            ATw = atp.tile([128, 32, 64], bf16, tag="ATw")
            for E, AT in [(EH, ATh), (EW, ATw)]:
                for p in range(32):
                    pst = psT.tile([128, 64], edt, tag="pt")
                    nc.tensor.transpose(
                        pst[:],
                        E[:, 2 * p:2 * p + 2, :].rearrange("p a b -> p (a b)"),
                        eident[:])
                    nc.scalar.activation(AT[:, p, :], pst[:], AF.Copy)
        else:
            ATh = atp.tile([64, 64, 64], bf16, tag="ATh")
            ATw = atp.tile([64, 64, 64], bf16, tag="ATw")
            for E, AT in [(EH, ATh), (EW, ATw)]:
                for u in range(64):
                    pst = psT.tile([64, 64], edt, tag="pt")
                    nc.tensor.transpose(pst[:], E[:, u, :], eident[:])
                    nc.scalar.activation(AT[:, u, :], pst[:], AF.Copy)
        esB.close()
        # ---- apply: w-phase (out_h) then h-phase (out_w)
        with contextlib.ExitStack() as esC:
            vtp = esC.enter_context(tc.tile_pool(name="vtp", bufs=6))
            psV = esC.enter_context(tc.tile_pool(name="psV", bufs=3, space="PSUM"))
            psO = esC.enter_context(tc.tile_pool(name="psO", bufs=4, space="PSUM"))
            for phase in range(2):  # 0: w-phase (per-column), 1: h-phase (per-row)
                AT = ATh if phase == 0 else ATw
                for c4 in range(16):  # chunks of 4 columns/rows
                    vts = []
                    if V_PAIR:
                        for pr in range(2):
                            u0 = c4 * 4 + pr * 2
                            pv = psV.tile([128, 512], f32, tag="pv")
                            for cb in range(4):
                                rv = _rows(SRC[cb])
                                # stationary operand needs one contiguous
                                # free dim: stage the (pair, 64) slab first
                                vstg = vtp.tile([128, 2, 64], bf16, tag="vstg")
                                if phase == 0:
                                    nc.vector.tensor_copy(
                                        vstg[:],
                                        rv[:, 1:65, u0:u0 + 2].rearrange(
                                            "p r w -> p w r"))
                                else:
                                    nc.vector.tensor_copy(
                                        vstg[:], rv[:, 1 + u0:3 + u0, 0:64])
                                nc.tensor.matmul(
                                    pv[:], vstg[:].rearrange("p a b -> p (a b)"),
                                    C['wv'][cb][:],
                                    start=(cb == 0), stop=(cb == 3))
                            vt = vtp.tile([128, 512], bf16, tag="vt")
                            nc.scalar.activation(vt[:], pv[:], AF.Copy)
                            vts.append(vt)
                    else:
                        for i in range(4):
                            u = c4 * 4 + i
                            pv = psV.tile([64, 512], f32, tag="pv")
                            for cb in range(4):
                                rv = _rows(SRC[cb])
                                lhsT = (rv[:, 1:65, u] if phase == 0
                                        else rv[:, 1 + u, 0:64])
                                nc.tensor.matmul(pv[:], lhsT, C['wv'][cb][:],
                                                 start=(cb == 0), stop=(cb == 3))
                            vt = vtp.tile([64, 512], bf16, tag="vt")
                            nc.scalar.activation(vt[:], pv[:], AF.Copy)
                            vts.append(vt)
                    for cbo in range(4):
                        po = psO.tile([128, 4, 64], f32, tag="po")
                        for i in range(4):
                            if V_PAIR:
                                pr, par = i // 2, (i % 2) * 64
                                lhsT = vts[pr][par:par + 64,
                                               cbo * 128:(cbo + 1) * 128]
                                rhs = AT[par:par + 64, c4 * 2 + pr, :]
                            else:
                                lhsT = vts[i][:, cbo * 128:(cbo + 1) * 128]
                                rhs = AT[:, c4 * 4 + i, :]
                            nc.tensor.matmul(po[:, i, :], lhsT, rhs,
                                             start=True, stop=True)
                        rvD = _rows(DST[cbo])
                        rvS = _rows(SRC[cbo])
                        if phase == 0:
                            o_sl = rvD[:, 1:65, c4 * 4:c4 * 4 + 4].rearrange(
                                "p h w -> p w h")
                            i_sl = rvS[:, 1:65, c4 * 4:c4 * 4 + 4].rearrange(
                                "p h w -> p w h")
                            nc.vector.scalar_tensor_tensor(
                                o_sl, po[:], gamma, i_sl, ALU.mult, ALU.add)
                        else:
                            o_sl = rvD[:, 1 + c4 * 4:5 + c4 * 4, 0:64]
                            nc.vector.scalar_tensor_tensor(
                                o_sl, po[:], gamma, o_sl, ALU.mult, ALU.add)
        # ---- + gamma * v_b (joint softmax sums to 1 across both branches)
        for cbo in range(4):
            o_in = _rows(DST[cbo])[:, 1:65, 0:64]
            nc.vector.tensor_scalar_add(o_in, o_in, C['gvb'][cbo][:])


_BUILD_CACHE = {}


def _get_nc(gamma):
    key = round(float(gamma), 12)
    if key not in _BUILD_CACHE:
        _BUILD_CACHE[key] = build(gamma, n_reps=1)
    return _BUILD_CACHE[key]


def kernel(**inputs):
    from concourse.bass_utils import run_bass_kernel_spmd
    inputs_np = {k: np.asarray(v) for k, v in inputs.items()}
    dev, gamma = host_prep(inputs_np)
    nc = _get_nc(gamma)
    xbf = inputs_np['x'].astype(ml_dtypes.bfloat16)
    in_maps = []
    for core in range(8):
        m = dict(dev)
        m['x'] = np.ascontiguousarray(xbf[core].reshape(*X_DEV_SHAPE))
        in_maps.append(m)
    res = run_bass_kernel_spmd(nc, in_maps, core_ids=list(range(8)))
    out = np.stack([r['y'].reshape(104, 64, 64) for r in res.results])
    return out.astype(np.float32)
